# revision 5
# baseline (speedup 1.0000x reference)
"""MoE grouped-GEMM kernel for Trainium2 (8 NeuronCores, expert-parallel).

Problem: x [16384, 1024] fp16, expert_indices [16384] int32 (0..7),
weights [8, 1024, 4096] fp16. Output: fp16 [16384, 4096] in sorted-token
order (stable sort by expert), fp32 accumulation.

Sharding: the host performs the argsort/bincount dispatch (that IS the
sharding step) and gives core e the tokens routed to expert e as a
pre-transposed xT [K, Mpad] fp16 block plus that expert's weights
[K, N]. Every core runs the identical dense-GEMM program (token counts
padded to a common multiple of 128), so a single SPMD NEFF drives all 8
cores with no device-side collectives. The host concatenates the
per-expert output blocks, which is exactly sorted-token order.
"""

import numpy as np

_NCORES = 8


def _build_program(T, K, N):
    """Dense GEMM per core: out[Mpad, N] = xT.T @ w, fp32 PSUM accumulation.

    Layout per core:
      xT [K, Mpad] fp16  (x pre-transposed on host so K lands on partitions)
      w  [K, N]   fp16
      out [Mpad, N] fp16, Mpad = T*128

    PE mapping: stationary lhsT = xT k-tile [128, 128], moving rhs = w
    [128, 512] slice, PSUM pieces of [128, 512] fp32 (1 bank, bufs=8)
    accumulated over K/128 k-tiles; DVE casts each piece to fp16 and the
    sync/scalar rails alternate stores.

    The input side is DMA-TRIGGER-rate limited, not bandwidth limited:
    each dma_start costs ~650ns on its issuing sequencer while the
    descriptors fan out across all 16 HW queues quickly. So inputs use
    FEW, LARGE transfers, the critical h=0 weight strips issue first on
    the sync rail (9 triggers), and x plus the h=1 strips ride the
    otherwise-idle gpsimd rail. The first RT tiles are processed JOINTLY
    in 512-col phases so each fresh weight strip feeds RT matmuls — the
    first pass is the only phase where every matmul needs first-use
    weight bytes.
    """
    from concourse import bacc, bass, tile
    import concourse.mybir as mybir
    from concourse.vector_clock import ScopedClock

    class _FastExitTC(tile.TileContext):
        # The stock exit path is drain -> barrier -> sem clears ->
        # barrier (~5us). The clears and second barrier only matter if
        # the NEFF is re-executed with warm semaphore state; this kernel
        # compiles a fresh NEFF per call and executes it once, so end
        # after the first barrier.
        def _drain_and_barrier(self, tick_clock, wait_clock):
            drain_inst = self.nc.sync.drain()
            wait_clock.add_sem_waits(
                drain_inst.ins, ScopedClock({None: tick_clock.global_clock})
            )
            self.nc.all_engine_barrier()
            popped = self.nc._tile_sem_poison_stack.pop()
            assert popped is self._sem_poison

    f16 = mybir.dt.float16
    f32 = mybir.dt.float32
    Mpad = T * 128
    KT = K // 128            # k-tiles (contraction)
    NB = 512                 # matmul moving width = PSUM piece width (1 bank)
    NH = 2048                # w h-strip width
    nhalves = N // NH
    NS = N // NB             # 512-col slices across the full output width

    # Skip the ctor-time all-engine barrier (~3.4us of engine-arrival
    # stagger plus serialization before the first DMA can issue). All
    # cross-engine ordering in this kernel goes through semaphores, which
    # the runtime zeroes at NEFF load, and the NEFF runs exactly once per
    # compile — the barrier only guards warm-state reuse. The patch is
    # restored before TileContext exit, which still emits its barrier.
    _orig_aeb = bass.Bass.all_engine_barrier
    bass.Bass.all_engine_barrier = lambda self, *a, **k: None
    try:
        nc = bacc.Bacc(
            "TRN2",
            target_bir_lowering=False,
            debug=False,
            num_devices=_NCORES,
            # pure data-parallel SPMD: no instruction reads the core id
            enable_partition_id=False,
        )
    finally:
        bass.Bass.all_engine_barrier = _orig_aeb
    xT = nc.dram_tensor("xT", [K, Mpad], f16, kind="ExternalInput").ap()
    w = nc.dram_tensor("w", [K, N], f16, kind="ExternalInput").ap()
    out = nc.dram_tensor("out", [Mpad, N], f16, kind="ExternalOutput").ap()

    RT = min(8, T)           # tiles processed jointly during the ramp

    with _FastExitTC(nc) as tc:
        with (
            tc.tile_pool(name="xw", bufs=1) as xw,
            tc.tile_pool(name="op", bufs=8) as op,
            tc.tile_pool(name="pp", bufs=8, space=bass.MemorySpace.PSUM) as pp,
        ):
            # PE clock-gate warm-up: matmuls on memset tiles issued
            # during the initial DMA wait so the HAM un-throttles (1.2 ->
            # 2.4GHz takes ~3.4us of sustained PE activity) before the
            # first real matmul. gpsimd exits the entry butterfly
            # earliest, so its memsets unblock the dummy burst soonest.
            zs = xw.tile([128, 128], f16, tag="zstat")
            zm = xw.tile([128, NB], f16, tag="zmov")
            nc.gpsimd.memset(zs[:], 0.0)
            nc.gpsimd.memset(zm[:], 0.0)
            pwarm = pp.tile([128, NB], f32, tag="ps")
            for i in range(7):
                nc.tensor.matmul(
                    pwarm[:], zs[:], zm[:], start=(i == 0), stop=(i == 6)
                )

            # Input DMAs. Sync rail (critical path): the h=0 weight
            # strips, k=0 split so the very first matmul's deps are one
            # small transfer. gpsimd rail (idle otherwise): the x strips
            # in first-use order, then the h=1 weight strips. Whole x and
            # w stay SBUF-resident (~107KB/partition with output bufs).
            w0c = xw.tile([128, NB], f16, tag="w0c")
            nc.sync.dma_start(w0c[:], w[0:128, 0:NB])
            ws0 = [None] * KT
            for k in range(1, KT):
                wt = xw.tile([128, NH], f16, tag=f"w{k}h0")
                nc.sync.dma_start(wt[:], w[k * 128 : (k + 1) * 128, 0:NH])
                ws0[k] = wt
            w0r = xw.tile([128, NH - NB], f16, tag="w0r")
            nc.sync.dma_start(w0r[:], w[0:128, NB:NH])

            xheads = []
            xlate = []
            for k in range(KT):
                xh = xw.tile([128, RT * 128], f16, tag=f"xh{k}")
                nc.gpsimd.dma_start(xh[:], xT[k * 128 : (k + 1) * 128, 0 : RT * 128])
                xheads.append(xh)
            if T > RT:
                for k in range(KT):
                    xl = xw.tile([128, (T - RT) * 128], f16, tag=f"xl{k}")
                    nc.gpsimd.dma_start(
                        xl[:], xT[k * 128 : (k + 1) * 128, RT * 128 : Mpad]
                    )
                    xlate.append(xl)
            ws1 = [None] * KT
            for h in range(1, nhalves):
                for k in range(KT):
                    wt = xw.tile([128, NH], f16, tag=f"w{k}h{h}")
                    nc.gpsimd.dma_start(
                        wt[:], w[k * 128 : (k + 1) * 128, h * NH : (h + 1) * NH]
                    )
                    ws1[k] = wt

            def lhs_for(k, t):
                if t < RT:
                    return xheads[k][:, t * 128 : (t + 1) * 128]
                return xlate[k][:, (t - RT) * 128 : (t - RT + 1) * 128]

            def rhs_for(k, s):
                # s is the 512-col slice index within the full N width
                if s < NH // NB:
                    if k == 0:
                        return w0c[:] if s == 0 else w0r[:, (s - 1) * NB : s * NB]
                    return ws0[k][:, s * NB : (s + 1) * NB]
                return ws1[k][:, (s - NH // NB) * NB : (s - NH // NB + 1) * NB]

            # Output chunks alternate rails (either alone barely keeps
            # up); parity arranged so the very last chunk — which gates
            # the exit drain — rides the faster sync rail. Pieces: RT*NS
            # during/after ramp ordering + (T-RT)*NS steady; the last
            # piece is stored as two half chunks.
            n_chunks = T * NS + 1
            chunk_i = [0]

            def store(ps, t, col0, nq):
                for q in range(NB // nq):
                    ot = op.tile([128, NB], f16, tag="ot")
                    nc.vector.tensor_copy(
                        ot[:, :nq], ps[:, q * nq : (q + 1) * nq]
                    )
                    eng = (
                        nc.sync
                        if (n_chunks - 1 - chunk_i[0]) % 2 == 0
                        else nc.scalar
                    )
                    chunk_i[0] += 1
                    c0 = col0 + q * nq
                    eng.dma_start(
                        out[t * 128 : (t + 1) * 128, c0 : c0 + nq], ot[:, :nq]
                    )

            # Ramp: tiles 0..RT-1 jointly, one NB-wide phase per h=0
            # slice — each fresh weight slice feeds RT matmuls.
            for s in range(NH // NB):
                pss = [
                    pp.tile([128, NB], f32, tag="ps", name=f"psr{s}_{i}")
                    for i in range(RT)
                ]
                for k in range(KT):
                    rhs = rhs_for(k, s)
                    for i in range(RT):
                        nc.tensor.matmul(
                            pss[i][:],
                            lhs_for(k, i),
                            rhs,
                            start=(k == 0),
                            stop=(k == KT - 1),
                        )
                for i in range(RT):
                    store(pss[i], i, s * NB, NB)

            # Remaining slices for the ramp tiles, then the late tiles
            # over the full width. One NB piece at a time, 8-deep ring.
            def piece(t, s, last):
                ps = pp.tile([128, NB], f32, tag="ps")
                for k in range(KT):
                    nc.tensor.matmul(
                        ps[:],
                        lhs_for(k, t),
                        rhs_for(k, s),
                        start=(k == 0),
                        stop=(k == KT - 1),
                    )
                store(ps, t, s * NB, NB // 2 if last else NB)

            for t in range(RT):
                for s in range(NH // NB, NS):
                    piece(t, s, last=(T == RT and t == T - 1 and s == NS - 1))
            for t in range(RT, T):
                for s in range(NS):
                    piece(t, s, last=(t == T - 1 and s == NS - 1))
    nc.compile()
    return nc


# test.py reads these after a call for timing/trace introspection
last_results = None


def kernel(x, expert_indices, weights):
    x = np.asarray(x)
    ei = np.asarray(expert_indices)
    w = np.asarray(weights)
    M, K = x.shape
    E, K2, N = w.shape
    assert K == K2 and E == _NCORES

    counts = np.bincount(ei, minlength=E)
    T = max(1, -(-int(counts.max()) // 128))
    Mpad = T * 128
    order = np.argsort(ei, kind="stable")
    x_sorted = x[order]
    offs = np.zeros(E + 1, dtype=np.int64)
    np.cumsum(counts, out=offs[1:])

    in_maps = []
    for e in range(E):
        blk = x_sorted[offs[e] : offs[e + 1]]
        xeT = np.zeros((K, Mpad), dtype=np.float16)
        xeT[:, : blk.shape[0]] = blk.T
        in_maps.append({"xT": xeT, "w": np.ascontiguousarray(w[e])})

    nc = _build_program(T, K, N)

    from concourse.bass_utils import run_bass_kernel_spmd

    res = run_bass_kernel_spmd(nc, in_maps, list(range(E)))
    global last_results
    last_results = res

    out = np.empty((M, N), dtype=np.float16)
    for e in range(E):
        out[offs[e] : offs[e + 1]] = res.results[e]["out"][: counts[e]]
    return out


# revision 9
# speedup vs baseline: 1.0250x; 1.0250x over previous
"""MoE grouped-GEMM kernel for Trainium2 (8 NeuronCores, expert-parallel).

Problem: x [16384, 1024] fp16, expert_indices [16384] int32 (0..7),
weights [8, 1024, 4096] fp16. Output: fp16 [16384, 4096] in sorted-token
order (stable sort by expert), fp32 accumulation.

Sharding: the host performs the argsort/bincount dispatch (that IS the
sharding step) and gives core e the tokens routed to expert e as a
pre-transposed xT [K, Mpad] fp16 block plus that expert's weights
[K, N]. Every core runs the identical dense-GEMM program (token counts
padded to a common multiple of 128), so a single SPMD NEFF drives all 8
cores with no device-side collectives. The host concatenates the
per-expert output blocks, which is exactly sorted-token order.
"""

import numpy as np

_NCORES = 8


def _build_program(T, K, N):
    """Dense GEMM per core: out[Mpad, N] = xT.T @ w, fp32 PSUM accumulation.

    Layout per core:
      xT [K, Mpad] fp16  (x pre-transposed on host so K lands on partitions)
      w  [K, N]   fp16
      out [Mpad, N] fp16, Mpad = T*128

    PE mapping: stationary lhsT = xT k-tile [128, 128], moving rhs = w
    [128, 512] slice, PSUM pieces of [128, 512] fp32 (1 bank, bufs=8)
    accumulated over K/128 k-tiles; DVE casts each piece to fp16 and the
    sync/scalar rails alternate stores.

    Measured input constraints this schedule is built around: each
    dma_start costs ~650ns on its issuing sequencer (and blocks on DGE
    ring space), and aggregate input delivery is only ~150-230GB/s over
    the opening 30us — so the h=0 weights (4MB) cannot all land before
    ~25us no matter what. The ramp therefore runs K-MAJOR over 4 tiles x
    2 slices (8 open PSUM pieces): each arriving per-k half-strip of w
    unlocks 8 matmuls (~1.7us of PE work), matching the ~1.3-2us
    half-strip arrival cadence. The h=0 halves ride sync+scalar (8
    triggers each), x and the h=1 strips ride the otherwise-idle gpsimd
    rail (h=1 split with sync), and outputs alternate sync/scalar.
    """
    from concourse import bacc, bass, tile
    import concourse.mybir as mybir
    from concourse.vector_clock import ScopedClock

    class _FastExitTC(tile.TileContext):
        # The stock exit path is drain -> barrier -> sem clears ->
        # barrier (~5us). The clears and second barrier only matter if
        # the NEFF is re-executed with warm semaphore state; this kernel
        # compiles a fresh NEFF per call and executes it once, so end
        # after the first barrier.
        def _drain_and_barrier(self, tick_clock, wait_clock):
            drain_inst = self.nc.sync.drain()
            wait_clock.add_sem_waits(
                drain_inst.ins, ScopedClock({None: tick_clock.global_clock})
            )
            self.nc.all_engine_barrier()
            popped = self.nc._tile_sem_poison_stack.pop()
            assert popped is self._sem_poison

    f16 = mybir.dt.float16
    f32 = mybir.dt.float32
    Mpad = T * 128
    KT = K // 128            # k-tiles (contraction)
    NB = 512                 # matmul moving width = PSUM piece width (1 bank)
    NH = 2048                # h=0 strip width
    nhalves = N // NH
    NS = N // NB             # 512-col slices across the full output width
    HS = NH // NB            # slices per h-strip

    # Skip the ctor-time all-engine barrier (~3.4us of engine-arrival
    # stagger plus serialization before the first DMA can issue). All
    # cross-engine ordering in this kernel goes through semaphores, which
    # the runtime zeroes at NEFF load, and the NEFF runs exactly once per
    # compile — the barrier only guards warm-state reuse. The patch is
    # restored before TileContext exit, which still emits its barrier.
    _orig_aeb = bass.Bass.all_engine_barrier
    bass.Bass.all_engine_barrier = lambda self, *a, **k: None
    try:
        nc = bacc.Bacc(
            "TRN2",
            target_bir_lowering=False,
            debug=False,
            num_devices=_NCORES,
            # pure data-parallel SPMD: no instruction reads the core id
            enable_partition_id=False,
        )
    finally:
        bass.Bass.all_engine_barrier = _orig_aeb
    xT = nc.dram_tensor("xT", [K, Mpad], f16, kind="ExternalInput").ap()
    w = nc.dram_tensor("w", [K, N], f16, kind="ExternalInput").ap()
    out = nc.dram_tensor("out", [Mpad, N], f16, kind="ExternalOutput").ap()

    RT = min(4, T)           # tiles swept jointly during the ramp
    TE = min(8, T)           # ramp + early tiles (x delivered before late)

    with _FastExitTC(nc) as tc:
        with (
            tc.tile_pool(name="xw", bufs=1) as xw,
            tc.tile_pool(name="op", bufs=8) as op,
            tc.tile_pool(name="pp", bufs=8, space=bass.MemorySpace.PSUM) as pp,
        ):
            # PE clock-gate warm-up: matmuls on memset tiles issued
            # during the initial DMA wait so the HAM un-throttles (1.2 ->
            # 2.4GHz takes ~3.4us of sustained PE activity) before the
            # first real matmul. gpsimd exits the entry butterfly
            # earliest, so its memsets unblock the dummy burst soonest.
            zs = xw.tile([128, 128], f16, tag="zstat")
            zm = xw.tile([128, NB], f16, tag="zmov")
            nc.gpsimd.memset(zs[:], 0.0)
            nc.gpsimd.memset(zm[:], 0.0)
            pwarm = pp.tile([128, NB], f32, tag="ps")
            for i in range(7):
                nc.tensor.matmul(
                    pwarm[:], zs[:], zm[:], start=(i == 0), stop=(i == 6)
                )

            # Input DMAs (see module docstring for the rail budget).
            wlo = [None] * KT        # h=0 cols [0, NH/2)    — sync
            whi = [None] * KT        # h=0 cols [NH/2, NH)   — scalar
            ws1 = [None] * KT        # h=1 strips [128, NH]  — sync + gpsimd
            HW = NH // 2
            for k in range(KT):
                wt = xw.tile([128, HW], f16, tag=f"wlo{k}")
                nc.sync.dma_start(wt[:], w[k * 128 : (k + 1) * 128, 0:HW])
                wlo[k] = wt
            for k in range(KT):
                wt = xw.tile([128, HW], f16, tag=f"whi{k}")
                nc.scalar.dma_start(wt[:], w[k * 128 : (k + 1) * 128, HW:NH])
                whi[k] = wt
            xheads = []
            xearly = []
            xlate = []
            for k in range(KT):
                xh = xw.tile([128, RT * 128], f16, tag=f"xh{k}")
                nc.gpsimd.dma_start(xh[:], xT[k * 128 : (k + 1) * 128, 0 : RT * 128])
                xheads.append(xh)
            if TE > RT:
                for k in range(KT):
                    xe = xw.tile([128, (TE - RT) * 128], f16, tag=f"xe{k}")
                    nc.gpsimd.dma_start(
                        xe[:], xT[k * 128 : (k + 1) * 128, RT * 128 : TE * 128]
                    )
                    xearly.append(xe)
            if nhalves > 1:
                for k in range(KT):
                    wt = xw.tile([128, NH], f16, tag=f"w{k}h1")
                    nc.gpsimd.dma_start(
                        wt[:], w[k * 128 : (k + 1) * 128, NH : 2 * NH]
                    )
                    ws1[k] = wt
            if T > TE:
                for k in range(KT):
                    xl = xw.tile([128, (T - TE) * 128], f16, tag=f"xl{k}")
                    nc.gpsimd.dma_start(
                        xl[:], xT[k * 128 : (k + 1) * 128, TE * 128 : Mpad]
                    )
                    xlate.append(xl)

            def lhs_for(k, t):
                if t < RT:
                    return xheads[k][:, t * 128 : (t + 1) * 128]
                if t < TE:
                    return xearly[k][:, (t - RT) * 128 : (t - RT + 1) * 128]
                return xlate[k][:, (t - TE) * 128 : (t - TE + 1) * 128]

            def rhs_for(k, s):
                # s is the 512-col slice index within the full N width
                if s < HS:
                    half, n = (wlo, s) if s < HS // 2 else (whi, s - HS // 2)
                    return half[k][:, n * NB : (n + 1) * NB]
                return ws1[k][:, (s - HS) * NB : (s - HS + 1) * NB]

            # Output chunks alternate rails (either alone barely keeps
            # up); parity arranged so the very last chunk — which gates
            # the exit drain — rides the faster sync rail. The last piece
            # is stored as two half chunks.
            n_chunks = T * NS + 1
            chunk_i = [0]

            def store(ps, t, col0, nq):
                for q in range(NB // nq):
                    ot = op.tile([128, NB], f16, tag="ot")
                    nc.vector.tensor_copy(
                        ot[:, :nq], ps[:, q * nq : (q + 1) * nq]
                    )
                    eng = (
                        nc.sync
                        if (n_chunks - 1 - chunk_i[0]) % 2 == 0
                        else nc.scalar
                    )
                    chunk_i[0] += 1
                    c0 = col0 + q * nq
                    eng.dma_start(
                        out[t * 128 : (t + 1) * 128, c0 : c0 + nq], ot[:, :nq]
                    )

            # Ramp: K-MAJOR sweeps over tiles 0..RT-1, two slices per
            # sweep — every arriving per-k half-strip unlocks RT*2
            # matmuls, so the PE keeps pace with first-use delivery.
            for sw in range(HS // 2):
                pss = [
                    pp.tile([128, NB], f32, tag="ps", name=f"psr{sw}_{j}")
                    for j in range(2 * RT)
                ]
                for k in range(KT):
                    for si in range(2):
                        rhs = rhs_for(k, sw * 2 + si)
                        for i in range(RT):
                            nc.tensor.matmul(
                                pss[si * RT + i][:],
                                lhs_for(k, i),
                                rhs,
                                start=(k == 0),
                                stop=(k == KT - 1),
                            )
                for si in range(2):
                    for i in range(RT):
                        store(pss[si * RT + i], i, (sw * 2 + si) * NB, NB)

            # Steady state: one NB piece at a time, 8-deep psum ring.
            def piece(t, s, last):
                ps = pp.tile([128, NB], f32, tag="ps")
                for k in range(KT):
                    nc.tensor.matmul(
                        ps[:],
                        lhs_for(k, t),
                        rhs_for(k, s),
                        start=(k == 0),
                        stop=(k == KT - 1),
                    )
                store(ps, t, s * NB, NB // 2 if last else NB)

            # Early tiles over the h=0 width (weights resident by now).
            for t in range(RT, TE):
                for s in range(HS):
                    piece(t, s, last=False)
            # The h=1 half for tiles 0..TE-1: K-MAJOR sweeps over TE
            # tiles, one slice each — the h=1 strips stream in on the
            # gpsimd rail behind the x strips, and a sweep only needs
            # strip k at its k-visit, ~1.7us * k into the sweep.
            for s in range(HS, NS):
                pss = [
                    pp.tile([128, NB], f32, tag="ps", name=f"psh{s}_{i}")
                    for i in range(TE)
                ]
                for k in range(KT):
                    rhs = rhs_for(k, s)
                    for i in range(TE):
                        nc.tensor.matmul(
                            pss[i][:],
                            lhs_for(k, i),
                            rhs,
                            start=(k == 0),
                            stop=(k == KT - 1),
                        )
                for i in range(TE):
                    store(
                        pss[i],
                        i,
                        s * NB,
                        NB // 2 if (T == TE and s == NS - 1 and i == TE - 1) else NB,
                    )
            # Late tiles across the full width.
            for t in range(TE, T):
                for s in range(NS):
                    piece(t, s, last=(t == T - 1 and s == NS - 1))
    nc.compile()
    return nc


# test.py reads these after a call for timing/trace introspection
last_results = None


def kernel(x, expert_indices, weights):
    x = np.asarray(x)
    ei = np.asarray(expert_indices)
    w = np.asarray(weights)
    M, K = x.shape
    E, K2, N = w.shape
    assert K == K2 and E == _NCORES

    counts = np.bincount(ei, minlength=E)
    T = max(1, -(-int(counts.max()) // 128))
    Mpad = T * 128
    order = np.argsort(ei, kind="stable")
    x_sorted = x[order]
    offs = np.zeros(E + 1, dtype=np.int64)
    np.cumsum(counts, out=offs[1:])

    in_maps = []
    for e in range(E):
        blk = x_sorted[offs[e] : offs[e + 1]]
        xeT = np.zeros((K, Mpad), dtype=np.float16)
        xeT[:, : blk.shape[0]] = blk.T
        in_maps.append({"xT": xeT, "w": np.ascontiguousarray(w[e])})

    nc = _build_program(T, K, N)

    from concourse.bass_utils import run_bass_kernel_spmd

    res = run_bass_kernel_spmd(nc, in_maps, list(range(E)))
    global last_results
    last_results = res

    out = np.empty((M, N), dtype=np.float16)
    for e in range(E):
        out[offs[e] : offs[e + 1]] = res.results[e]["out"][: counts[e]]
    return out


# revision 12
# speedup vs baseline: 1.0514x; 1.0258x over previous
"""MoE grouped-GEMM kernel for Trainium2 (8 NeuronCores, expert-parallel).

Problem: x [16384, 1024] fp16, expert_indices [16384] int32 (0..7),
weights [8, 1024, 4096] fp16. Output: fp16 [16384, 4096] in sorted-token
order (stable sort by expert), fp32 accumulation.

Sharding: the host performs the argsort/bincount dispatch (that IS the
sharding step) and gives core e the tokens routed to expert e as a
pre-transposed xT [K, Mpad] fp16 block plus that expert's weights
[K, N]. Every core runs the identical dense-GEMM program (token counts
padded to a common multiple of 128), so a single SPMD NEFF drives all 8
cores with no device-side collectives. The host concatenates the
per-expert output blocks, which is exactly sorted-token order.
"""

import numpy as np

_NCORES = 8


def _build_program(T, K, N):
    """Dense GEMM per core: out[Mpad, N] = xT.T @ w, fp32 PSUM accumulation.

    Layout per core:
      xT [K, Mpad] fp16  (x pre-transposed on host so K lands on partitions)
      w  [K, N]   fp16
      out [Mpad, N] fp16, Mpad = T*128

    PE mapping: stationary lhsT = xT k-tile [128, 128], moving rhs = w
    [128, 512] slice, PSUM pieces of [128, 512] fp32 (1 bank, bufs=8)
    accumulated over K/128 k-tiles; DVE casts each piece to fp16 and the
    sync/scalar rails alternate stores.

    Measured input constraints this schedule is built around: each
    dma_start costs ~650ns on its issuing sequencer (and blocks on DGE
    ring space), and aggregate input delivery is only ~150-230GB/s over
    the opening 30us — so the h=0 weights (4MB) cannot all land before
    ~25us no matter what. The ramp therefore runs K-MAJOR over 4 tiles x
    2 slices (8 open PSUM pieces): each arriving per-k half-strip of w
    unlocks 8 matmuls (~1.7us of PE work), matching the ~1.3-2us
    half-strip arrival cadence. The h=0 halves ride sync+scalar (8
    triggers each), x and the h=1 strips ride the otherwise-idle gpsimd
    rail (h=1 split with sync), and outputs alternate sync/scalar.
    """
    from concourse import bacc, bass, tile
    import concourse.mybir as mybir
    from concourse.vector_clock import ScopedClock

    class _FastExitTC(tile.TileContext):
        # The stock exit path is drain -> barrier -> sem clears ->
        # barrier (~5us). The clears and second barrier only matter if
        # the NEFF is re-executed with warm semaphore state; this kernel
        # compiles a fresh NEFF per call and executes it once, so end
        # after the first barrier.
        def _drain_and_barrier(self, tick_clock, wait_clock):
            drain_inst = self.nc.sync.drain()
            wait_clock.add_sem_waits(
                drain_inst.ins, ScopedClock({None: tick_clock.global_clock})
            )
            self.nc.all_engine_barrier()
            popped = self.nc._tile_sem_poison_stack.pop()
            assert popped is self._sem_poison

    f16 = mybir.dt.float16
    f32 = mybir.dt.float32
    Mpad = T * 128
    KT = K // 128            # k-tiles (contraction)
    NB = 512                 # matmul moving width = PSUM piece width (1 bank)
    NH = 2048                # h=0 strip width
    nhalves = N // NH
    NS = N // NB             # 512-col slices across the full output width
    HS = NH // NB            # slices per h-strip

    # Skip the ctor-time all-engine barrier (~3.4us of engine-arrival
    # stagger plus serialization before the first DMA can issue). All
    # cross-engine ordering in this kernel goes through semaphores, which
    # the runtime zeroes at NEFF load, and the NEFF runs exactly once per
    # compile — the barrier only guards warm-state reuse. The patch is
    # restored before TileContext exit, which still emits its barrier.
    _orig_aeb = bass.Bass.all_engine_barrier
    bass.Bass.all_engine_barrier = lambda self, *a, **k: None
    try:
        nc = bacc.Bacc(
            "TRN2",
            target_bir_lowering=False,
            debug=False,
            num_devices=_NCORES,
            # pure data-parallel SPMD: no instruction reads the core id
            enable_partition_id=False,
        )
    finally:
        bass.Bass.all_engine_barrier = _orig_aeb
    xT = nc.dram_tensor("xT", [K, Mpad], f16, kind="ExternalInput").ap()
    w = nc.dram_tensor("w", [K, N], f16, kind="ExternalInput").ap()
    out = nc.dram_tensor("out", [Mpad, N], f16, kind="ExternalOutput").ap()

    RT = min(4, T)           # tiles swept jointly during the ramp
    TE = min(8, T)           # ramp + early tiles (x delivered before late)

    with _FastExitTC(nc) as tc:
        with (
            tc.tile_pool(name="xw", bufs=1) as xw,
            tc.tile_pool(name="op", bufs=8) as op,
            tc.tile_pool(name="pp", bufs=8, space=bass.MemorySpace.PSUM) as pp,
        ):
            # PE clock-gate warm-up: matmuls on memset tiles issued
            # during the initial DMA wait so the HAM un-throttles (1.2 ->
            # 2.4GHz takes ~3.4us of sustained PE activity) before the
            # first real matmul. gpsimd exits the entry butterfly
            # earliest, so its memsets unblock the dummy burst soonest.
            zs = xw.tile([128, 128], f16, tag="zstat")
            zm = xw.tile([128, NB], f16, tag="zmov")
            nc.gpsimd.memset(zs[:], 0.0)
            nc.gpsimd.memset(zm[:], 0.0)
            pwarm = pp.tile([128, NB], f32, tag="ps")
            for i in range(7):
                nc.tensor.matmul(
                    pwarm[:], zs[:], zm[:], start=(i == 0), stop=(i == 6)
                )

            # Input DMAs (see module docstring for the rail budget).
            wlo = [None] * KT        # h=0 cols [0, NH/2)    — sync
            whi = [None] * KT        # h=0 cols [NH/2, NH)   — scalar
            ws1 = [None] * KT        # h=1 strips [128, NH]  — sync + gpsimd
            # Per-rail delivery is ~85GB/s (~3us per 256KB half-strip), so
            # consecutive k's halves alternate rails: two successive
            # k-visits of a ramp sweep are then fed by different rails in
            # parallel (effective ~1.5us per visit vs 1.73us of PE work).
            HW = NH // 2
            w0c = []
            for n in range(HW // NB):
                c = xw.tile([128, NB], f16, tag=f"w0c{n}")
                nc.sync.dma_start(c[:], w[0:128, n * NB : (n + 1) * NB])
                w0c.append(c)
            for k in range(1, KT):
                wt = xw.tile([128, HW], f16, tag=f"wlo{k}")
                eng = nc.sync if k % 2 == 0 else nc.scalar
                eng.dma_start(wt[:], w[k * 128 : (k + 1) * 128, 0:HW])
                wlo[k] = wt
            for k in range(KT):
                wt = xw.tile([128, HW], f16, tag=f"whi{k}")
                eng = nc.sync if k % 2 == 1 else nc.scalar
                eng.dma_start(wt[:], w[k * 128 : (k + 1) * 128, HW:NH])
                whi[k] = wt
            xheads = []
            xearly = []
            xlate = []
            for k in range(KT):
                xh = xw.tile([128, RT * 128], f16, tag=f"xh{k}")
                nc.gpsimd.dma_start(xh[:], xT[k * 128 : (k + 1) * 128, 0 : RT * 128])
                xheads.append(xh)
            if TE > RT:
                for k in range(KT):
                    xe = xw.tile([128, (TE - RT) * 128], f16, tag=f"xe{k}")
                    nc.gpsimd.dma_start(
                        xe[:], xT[k * 128 : (k + 1) * 128, RT * 128 : TE * 128]
                    )
                    xearly.append(xe)
            if nhalves > 1:
                for k in range(KT):
                    wt = xw.tile([128, NH], f16, tag=f"w{k}h1")
                    nc.gpsimd.dma_start(
                        wt[:], w[k * 128 : (k + 1) * 128, NH : 2 * NH]
                    )
                    ws1[k] = wt
            if T > TE:
                for k in range(KT):
                    xl = xw.tile([128, (T - TE) * 128], f16, tag=f"xl{k}")
                    nc.gpsimd.dma_start(
                        xl[:], xT[k * 128 : (k + 1) * 128, TE * 128 : Mpad]
                    )
                    xlate.append(xl)

            def lhs_for(k, t):
                if t < RT:
                    return xheads[k][:, t * 128 : (t + 1) * 128]
                if t < TE:
                    return xearly[k][:, (t - RT) * 128 : (t - RT + 1) * 128]
                return xlate[k][:, (t - TE) * 128 : (t - TE + 1) * 128]

            def rhs_for(k, s):
                # s is the 512-col slice index within the full N width
                if s < HS:
                    if k == 0 and s < HS // 2:
                        return w0c[s][:]
                    half, n = (wlo, s) if s < HS // 2 else (whi, s - HS // 2)
                    return half[k][:, n * NB : (n + 1) * NB]
                return ws1[k][:, (s - HS) * NB : (s - HS + 1) * NB]

            # Output chunks alternate rails (either alone barely keeps
            # up); parity arranged so the very last chunk — which gates
            # the exit drain — rides the faster sync rail. The last piece
            # is stored as two half chunks.
            n_chunks = T * NS + 1
            chunk_i = [0]

            def store(ps, t, col0, nq):
                for q in range(NB // nq):
                    ot = op.tile([128, NB], f16, tag="ot")
                    nc.vector.tensor_copy(
                        ot[:, :nq], ps[:, q * nq : (q + 1) * nq]
                    )
                    eng = (
                        nc.sync
                        if (n_chunks - 1 - chunk_i[0]) % 2 == 0
                        else nc.scalar
                    )
                    chunk_i[0] += 1
                    c0 = col0 + q * nq
                    eng.dma_start(
                        out[t * 128 : (t + 1) * 128, c0 : c0 + nq], ot[:, :nq]
                    )

            # Ramp: K-MAJOR sweeps over tiles 0..RT-1, two slices per
            # sweep — every arriving per-k half-strip unlocks RT*2
            # matmuls, so the PE keeps pace with first-use delivery.
            for sw in range(HS // 2):
                pss = [
                    pp.tile([128, NB], f32, tag="ps", name=f"psr{sw}_{j}")
                    for j in range(2 * RT)
                ]
                for k in range(KT):
                    for si in range(2):
                        rhs = rhs_for(k, sw * 2 + si)
                        for i in range(RT):
                            nc.tensor.matmul(
                                pss[si * RT + i][:],
                                lhs_for(k, i),
                                rhs,
                                start=(k == 0),
                                stop=(k == KT - 1),
                            )
                for si in range(2):
                    for i in range(RT):
                        store(pss[si * RT + i], i, (sw * 2 + si) * NB, NB)

            # Steady state: one NB piece at a time, 8-deep psum ring.
            def piece(t, s, last):
                ps = pp.tile([128, NB], f32, tag="ps")
                for k in range(KT):
                    nc.tensor.matmul(
                        ps[:],
                        lhs_for(k, t),
                        rhs_for(k, s),
                        start=(k == 0),
                        stop=(k == KT - 1),
                    )
                store(ps, t, s * NB, NB // 2 if last else NB)

            # Early tiles over the h=0 width (weights resident by now).
            for t in range(RT, TE):
                for s in range(HS):
                    piece(t, s, last=False)
            # The h=1 half for tiles 0..TE-1: K-MAJOR sweeps over TE
            # tiles, one slice each — the h=1 strips stream in on the
            # gpsimd rail behind the x strips, and a sweep only needs
            # strip k at its k-visit, ~1.7us * k into the sweep.
            for s in range(HS, NS):
                pss = [
                    pp.tile([128, NB], f32, tag="ps", name=f"psh{s}_{i}")
                    for i in range(TE)
                ]
                for k in range(KT):
                    rhs = rhs_for(k, s)
                    for i in range(TE):
                        nc.tensor.matmul(
                            pss[i][:],
                            lhs_for(k, i),
                            rhs,
                            start=(k == 0),
                            stop=(k == KT - 1),
                        )
                for i in range(TE):
                    store(
                        pss[i],
                        i,
                        s * NB,
                        NB // 2 if (T == TE and s == NS - 1 and i == TE - 1) else NB,
                    )
            # Late tiles across the full width.
            for t in range(TE, T):
                for s in range(NS):
                    piece(t, s, last=(t == T - 1 and s == NS - 1))
    nc.compile()
    return nc


# test.py reads these after a call for timing/trace introspection
last_results = None


def kernel(x, expert_indices, weights):
    x = np.asarray(x)
    ei = np.asarray(expert_indices)
    w = np.asarray(weights)
    M, K = x.shape
    E, K2, N = w.shape
    assert K == K2 and E == _NCORES

    counts = np.bincount(ei, minlength=E)
    T = max(1, -(-int(counts.max()) // 128))
    Mpad = T * 128
    order = np.argsort(ei, kind="stable")
    x_sorted = x[order]
    offs = np.zeros(E + 1, dtype=np.int64)
    np.cumsum(counts, out=offs[1:])

    in_maps = []
    for e in range(E):
        blk = x_sorted[offs[e] : offs[e + 1]]
        xeT = np.zeros((K, Mpad), dtype=np.float16)
        xeT[:, : blk.shape[0]] = blk.T
        in_maps.append({"xT": xeT, "w": np.ascontiguousarray(w[e])})

    nc = _build_program(T, K, N)

    from concourse.bass_utils import run_bass_kernel_spmd

    res = run_bass_kernel_spmd(nc, in_maps, list(range(E)))
    global last_results
    last_results = res

    out = np.empty((M, N), dtype=np.float16)
    for e in range(E):
        out[offs[e] : offs[e + 1]] = res.results[e]["out"][: counts[e]]
    return out


# revision 18
# speedup vs baseline: 1.0805x; 1.0277x over previous
"""MoE grouped-GEMM kernel for Trainium2 (8 NeuronCores, expert-parallel).

Problem: x [16384, 1024] fp16, expert_indices [16384] int32 (0..7),
weights [8, 1024, 4096] fp16. Output: fp16 [16384, 4096] in sorted-token
order (stable sort by expert), fp32 accumulation.

Sharding: the host performs the argsort/bincount dispatch (that IS the
sharding step) and gives core e the tokens routed to expert e as a
pre-transposed xT [K, Mpad] fp16 block plus that expert's weights
[K, N]. Every core runs the identical dense-GEMM program (token counts
padded to a common multiple of 128), so a single SPMD NEFF drives all 8
cores with no device-side collectives. The host concatenates the
per-expert output blocks, which is exactly sorted-token order.
"""

import numpy as np

_NCORES = 8


def _build_program(T, K, N, nloose=0):
    """Dense GEMM per core: out[Mpad, N] = xT.T @ w, fp32 PSUM accumulation.

    When nloose > 0, the core additionally computes nloose single-slice
    "loose" pieces outx[j] = xx[:, j].T @ wx[j] at the end — the host
    shatters the surplus tiles of oversubscribed experts into such
    pieces so every core runs exactly T whole tiles + nloose pieces
    (perfect load balance across the 8 cores instead of padding every
    core to the busiest expert's tile count).

    Layout per core:
      xT [K, Mpad] fp16  (x pre-transposed on host so K lands on partitions)
      w  [K, N]   fp16
      out [Mpad, N] fp16, Mpad = T*128

    PE mapping: stationary lhsT = xT k-tile [128, 128], moving rhs = w
    [128, 512] slice, PSUM pieces of [128, 512] fp32 (1 bank, bufs=8)
    accumulated over K/128 k-tiles; DVE casts each piece to fp16 and the
    sync/scalar rails alternate stores.

    Measured input constraints this schedule is built around: each
    dma_start costs ~650ns on its issuing sequencer (and blocks on DGE
    ring space), and aggregate input delivery is only ~150-230GB/s over
    the opening 30us — so the h=0 weights (4MB) cannot all land before
    ~25us no matter what. The ramp therefore runs K-MAJOR over 4 tiles x
    2 slices (8 open PSUM pieces): each arriving per-k half-strip of w
    unlocks 8 matmuls (~1.7us of PE work), matching the ~1.3-2us
    half-strip arrival cadence. The h=0 halves ride sync+scalar (8
    triggers each), x and the h=1 strips ride the otherwise-idle gpsimd
    rail (h=1 split with sync), and outputs alternate sync/scalar.
    """
    from concourse import bacc, bass, tile
    import concourse.mybir as mybir
    from concourse.vector_clock import ScopedClock

    class _FastExitTC(tile.TileContext):
        # The stock exit path is drain -> barrier -> sem clears ->
        # barrier (~5us). The clears and second barrier only matter if
        # the NEFF is re-executed with warm semaphore state; this kernel
        # compiles a fresh NEFF per call and executes it once, so end
        # after the first barrier.
        def _drain_and_barrier(self, tick_clock, wait_clock):
            drain_inst = self.nc.sync.drain()
            wait_clock.add_sem_waits(
                drain_inst.ins, ScopedClock({None: tick_clock.global_clock})
            )
            self.nc.all_engine_barrier()
            popped = self.nc._tile_sem_poison_stack.pop()
            assert popped is self._sem_poison

    f16 = mybir.dt.float16
    f32 = mybir.dt.float32
    Mpad = T * 128
    KT = K // 128            # k-tiles (contraction)
    NB = 512                 # matmul moving width = PSUM piece width (1 bank)
    NH = 2048                # h=0 strip width
    nhalves = N // NH
    NS = N // NB             # 512-col slices across the full output width
    HS = NH // NB            # slices per h-strip

    # Skip the ctor-time all-engine barrier (~3.4us of engine-arrival
    # stagger plus serialization before the first DMA can issue). All
    # cross-engine ordering in this kernel goes through semaphores, which
    # the runtime zeroes at NEFF load, and the NEFF runs exactly once per
    # compile — the barrier only guards warm-state reuse. The patch is
    # restored before TileContext exit, which still emits its barrier.
    _orig_aeb = bass.Bass.all_engine_barrier
    bass.Bass.all_engine_barrier = lambda self, *a, **k: None
    try:
        nc = bacc.Bacc(
            "TRN2",
            target_bir_lowering=False,
            debug=False,
            num_devices=_NCORES,
            # pure data-parallel SPMD: no instruction reads the core id
            enable_partition_id=False,
        )
    finally:
        bass.Bass.all_engine_barrier = _orig_aeb
    xT = nc.dram_tensor("xT", [K, Mpad], f16, kind="ExternalInput").ap()
    w = nc.dram_tensor("w", [K, N], f16, kind="ExternalInput").ap()
    out = nc.dram_tensor("out", [Mpad, N], f16, kind="ExternalOutput").ap()
    if nloose:
        xx = nc.dram_tensor("xx", [K, nloose * 128], f16, kind="ExternalInput").ap()
        wx = nc.dram_tensor("wx", [nloose * K, NB], f16, kind="ExternalInput").ap()
        outx = nc.dram_tensor(
            "outx", [nloose * 128, NB], f16, kind="ExternalOutput"
        ).ap()

    RT = min(4, T)           # tiles swept jointly during the ramp
    TE = min(8, T)           # ramp + early tiles (x delivered before late)

    with _FastExitTC(nc) as tc:
        with (
            tc.tile_pool(name="xw", bufs=1) as xw,
            tc.tile_pool(name="op", bufs=8) as op,
            tc.tile_pool(name="pp", bufs=8, space=bass.MemorySpace.PSUM) as pp,
        ):
            # PE clock-gate warm-up: matmuls on memset tiles issued
            # during the initial DMA wait so the HAM un-throttles (1.2 ->
            # 2.4GHz takes ~3.4us of sustained PE activity) before the
            # first real matmul. gpsimd exits the entry butterfly
            # earliest, so its memsets unblock the dummy burst soonest.
            zs = xw.tile([128, 128], f16, tag="zstat")
            zm = xw.tile([128, NB], f16, tag="zmov")
            nc.gpsimd.memset(zs[:], 0.0)
            nc.gpsimd.memset(zm[:], 0.0)
            pwarm = pp.tile([128, NB], f32, tag="ps")
            for i in range(7):
                nc.tensor.matmul(
                    pwarm[:], zs[:], zm[:], start=(i == 0), stop=(i == 6)
                )

            # Input DMAs (see module docstring for the rail budget).
            wlo = [None] * KT        # h=0 cols [0, NH/2)    — sync
            whi = [None] * KT        # h=0 cols [NH/2, NH)   — scalar
            ws1 = [None] * KT        # h=1 strips [128, NH]  — sync + gpsimd
            # Per-rail delivery is ~85GB/s (~3us per 256KB half-strip), so
            # consecutive k's halves alternate rails: two successive
            # k-visits of a ramp sweep are then fed by different rails in
            # parallel (effective ~1.5us per visit vs 1.73us of PE work).
            HW = NH // 2
            w0c = []
            for n in range(HW // NB):
                c = xw.tile([128, NB], f16, tag=f"w0c{n}")
                nc.sync.dma_start(c[:], w[0:128, n * NB : (n + 1) * NB])
                w0c.append(c)
            for k in range(1, KT):
                wt = xw.tile([128, HW], f16, tag=f"wlo{k}")
                eng = nc.sync if k % 2 == 0 else nc.scalar
                eng.dma_start(wt[:], w[k * 128 : (k + 1) * 128, 0:HW])
                wlo[k] = wt
            for k in range(KT):
                wt = xw.tile([128, HW], f16, tag=f"whi{k}")
                eng = nc.sync if k % 2 == 1 else nc.scalar
                eng.dma_start(wt[:], w[k * 128 : (k + 1) * 128, HW:NH])
                whi[k] = wt
            xheads = []
            xearly = []
            xlate = []
            for k in range(KT):
                xh = xw.tile([128, RT * 128], f16, tag=f"xh{k}")
                nc.gpsimd.dma_start(xh[:], xT[k * 128 : (k + 1) * 128, 0 : RT * 128])
                xheads.append(xh)
            if TE > RT:
                for k in range(KT):
                    xe = xw.tile([128, (TE - RT) * 128], f16, tag=f"xe{k}")
                    nc.gpsimd.dma_start(
                        xe[:], xT[k * 128 : (k + 1) * 128, RT * 128 : TE * 128]
                    )
                    xearly.append(xe)
            if nhalves > 1:
                for k in range(KT):
                    wt = xw.tile([128, NH], f16, tag=f"w{k}h1")
                    nc.gpsimd.dma_start(
                        wt[:], w[k * 128 : (k + 1) * 128, NH : 2 * NH]
                    )
                    ws1[k] = wt
            if T > TE:
                for k in range(KT):
                    xl = xw.tile([128, (T - TE) * 128], f16, tag=f"xl{k}")
                    nc.gpsimd.dma_start(
                        xl[:], xT[k * 128 : (k + 1) * 128, TE * 128 : Mpad]
                    )
                    xlate.append(xl)
            xxs = []
            wxs = []
            if nloose:
                for k in range(KT):
                    xt = xw.tile([128, nloose * 128], f16, tag=f"xx{k}")
                    nc.gpsimd.dma_start(
                        xt[:], xx[k * 128 : (k + 1) * 128, :]
                    )
                    xxs.append(xt)
                for j in range(nloose):
                    per_k = []
                    for k in range(KT):
                        wt = xw.tile([128, NB], f16, tag=f"wx{j}_{k}")
                        nc.gpsimd.dma_start(
                            wt[:],
                            wx[j * K + k * 128 : j * K + (k + 1) * 128, :],
                        )
                        per_k.append(wt)
                    wxs.append(per_k)

            def lhs_for(k, t):
                if t < RT:
                    return xheads[k][:, t * 128 : (t + 1) * 128]
                if t < TE:
                    return xearly[k][:, (t - RT) * 128 : (t - RT + 1) * 128]
                return xlate[k][:, (t - TE) * 128 : (t - TE + 1) * 128]

            def rhs_for(k, s):
                # s is the 512-col slice index within the full N width
                if s < HS:
                    if k == 0 and s < HS // 2:
                        return w0c[s][:]
                    half, n = (wlo, s) if s < HS // 2 else (whi, s - HS // 2)
                    return half[k][:, n * NB : (n + 1) * NB]
                return ws1[k][:, (s - HS) * NB : (s - HS + 1) * NB]

            # Output chunks alternate rails (either alone barely keeps
            # up); parity arranged so the very last chunk — which gates
            # the exit drain — rides the faster sync rail. The last piece
            # is stored as two half chunks.
            n_chunks = T * NS + nloose + 1
            chunk_i = [0]

            def store(ps, t, col0, nq, dst=out):
                for q in range(NB // nq):
                    ot = op.tile([128, NB], f16, tag="ot")
                    nc.vector.tensor_copy(
                        ot[:, :nq], ps[:, q * nq : (q + 1) * nq]
                    )
                    eng = (
                        nc.sync
                        if (n_chunks - 1 - chunk_i[0]) % 2 == 0
                        else nc.scalar
                    )
                    chunk_i[0] += 1
                    c0 = col0 + q * nq
                    eng.dma_start(
                        dst[t * 128 : (t + 1) * 128, c0 : c0 + nq], ot[:, :nq]
                    )

            # Ramp: K-MAJOR sweeps over tiles 0..RT-1, two slices per
            # sweep — every arriving per-k half-strip unlocks RT*2
            # matmuls, so the PE keeps pace with first-use delivery.
            for sw in range(HS // 2):
                pss = [
                    pp.tile([128, NB], f32, tag="ps", name=f"psr{sw}_{j}")
                    for j in range(2 * RT)
                ]
                for k in range(KT):
                    for si in range(2):
                        rhs = rhs_for(k, sw * 2 + si)
                        for i in range(RT):
                            nc.tensor.matmul(
                                pss[si * RT + i][:],
                                lhs_for(k, i),
                                rhs,
                                start=(k == 0),
                                stop=(k == KT - 1),
                            )
                for si in range(2):
                    for i in range(RT):
                        store(pss[si * RT + i], i, (sw * 2 + si) * NB, NB)

            # Steady state: one NB piece at a time, 8-deep psum ring.
            def piece(t, s, last):
                ps = pp.tile([128, NB], f32, tag="ps")
                for k in range(KT):
                    nc.tensor.matmul(
                        ps[:],
                        lhs_for(k, t),
                        rhs_for(k, s),
                        start=(k == 0),
                        stop=(k == KT - 1),
                    )
                store(ps, t, s * NB, NB // 2 if last else NB)

            # Early tiles over the h=0 width (weights resident by now).
            for t in range(RT, TE):
                for s in range(HS):
                    piece(t, s, last=False)
            # The h=1 half for tiles 0..TE-1: K-MAJOR sweeps over TE
            # tiles, one slice each — the h=1 strips stream in on the
            # gpsimd rail behind the x strips, and a sweep only needs
            # strip k at its k-visit, ~1.7us * k into the sweep.
            for s in range(HS, NS):
                pss = [
                    pp.tile([128, NB], f32, tag="ps", name=f"psh{s}_{i}")
                    for i in range(TE)
                ]
                for k in range(KT):
                    rhs = rhs_for(k, s)
                    for i in range(TE):
                        nc.tensor.matmul(
                            pss[i][:],
                            lhs_for(k, i),
                            rhs,
                            start=(k == 0),
                            stop=(k == KT - 1),
                        )
                for i in range(TE):
                    store(
                        pss[i],
                        i,
                        s * NB,
                        NB // 2
                        if (not nloose and T == TE and s == NS - 1 and i == TE - 1)
                        else NB,
                    )
            # Late tiles across the full width.
            for t in range(TE, T):
                for s in range(NS):
                    piece(t, s, last=(not nloose and t == T - 1 and s == NS - 1))
            # Loose pieces (host-shattered surplus tiles), the kernel tail.
            for j in range(nloose):
                ps = pp.tile([128, NB], f32, tag="ps", name=f"psx{j}")
                for k in range(KT):
                    nc.tensor.matmul(
                        ps[:],
                        xxs[k][:, j * 128 : (j + 1) * 128],
                        wxs[j][k][:],
                        start=(k == 0),
                        stop=(k == KT - 1),
                    )
                store(
                    ps, j, 0, NB // 2 if j == nloose - 1 else NB, dst=outx
                )
    nc.compile()
    return nc


# test.py reads these after a call for timing/trace introspection
last_results = None


def kernel(x, expert_indices, weights):
    x = np.asarray(x)
    ei = np.asarray(expert_indices)
    w = np.asarray(weights)
    M, K = x.shape
    E, K2, N = w.shape
    assert K == K2 and E == _NCORES

    counts = np.bincount(ei, minlength=E)
    order = np.argsort(ei, kind="stable")
    x_sorted = x[order]
    offs = np.zeros(E + 1, dtype=np.int64)
    np.cumsum(counts, out=offs[1:])

    NB = 512
    NS = N // NB
    tiles = [-(-int(c) // 128) for c in counts]
    total = sum(tiles)
    Tw = total // E
    loose_exp = [e for e in range(E) if counts[e] > Tw * 128]
    balanced = (
        Tw >= 8
        and min(tiles) >= Tw
        and max(counts) <= (Tw + 1) * 128
        and (len(loose_exp) * NS) % E == 0
    )

    from concourse.bass_utils import run_bass_kernel_spmd
    global last_results

    if not balanced:
        # Fallback: every core padded to the busiest expert's tile count.
        T = max(1, max(tiles))
        Mpad = T * 128
        in_maps = []
        for e in range(E):
            blk = x_sorted[offs[e] : offs[e + 1]]
            xeT = np.zeros((K, Mpad), dtype=np.float16)
            xeT[:, : blk.shape[0]] = blk.T
            in_maps.append({"xT": xeT, "w": np.ascontiguousarray(w[e])})
        nc = _build_program(T, K, N)
        res = run_bass_kernel_spmd(nc, in_maps, list(range(E)))
        last_results = res
        out = np.empty((M, N), dtype=np.float16)
        for e in range(E):
            out[offs[e] : offs[e + 1]] = res.results[e]["out"][: counts[e]]
        return out

    # Balanced partition: core e runs its expert's first Tw tiles plus
    # nloose loose (tile, slice) pieces shattered from the surplus tiles
    # of oversubscribed experts.
    pool = [(e, s) for e in loose_exp for s in range(NS)]
    nloose = len(pool) // E
    Mpad = Tw * 128
    in_maps = []
    for c in range(E):
        n_tok = min(int(counts[c]), Mpad)
        xeT = np.zeros((K, Mpad), dtype=np.float16)
        xeT[:, :n_tok] = x_sorted[offs[c] : offs[c] + n_tok].T
        xxm = np.zeros((K, nloose * 128), dtype=np.float16)
        wxm = np.empty((nloose * K, NB), dtype=np.float16)
        for j, (e, s) in enumerate(pool[c * nloose : (c + 1) * nloose]):
            r = int(counts[e]) - Mpad
            xxm[:, j * 128 : j * 128 + r] = x_sorted[
                offs[e] + Mpad : offs[e + 1]
            ].T
            wxm[j * K : (j + 1) * K] = w[e][:, s * NB : (s + 1) * NB]
        in_maps.append(
            {"xT": xeT, "w": np.ascontiguousarray(w[c]), "xx": xxm, "wx": wxm}
        )

    nc = _build_program(Tw, K, N, nloose=nloose)
    res = run_bass_kernel_spmd(nc, in_maps, list(range(E)))
    last_results = res

    out = np.empty((M, N), dtype=np.float16)
    for c in range(E):
        n_tok = min(int(counts[c]), Mpad)
        out[offs[c] : offs[c] + n_tok] = res.results[c]["out"][:n_tok]
        for j, (e, s) in enumerate(pool[c * nloose : (c + 1) * nloose]):
            r = int(counts[e]) - Mpad
            out[offs[e] + Mpad : offs[e + 1], s * NB : (s + 1) * NB] = (
                res.results[c]["outx"][j * 128 : j * 128 + r]
            )
    return out
